# revision 57
# baseline (speedup 1.0000x reference)
"""LowPassMSELoss Trainium2 kernel (v4: chunked DMA pipeline, bf16 PE path,
PE HAM warmup, DVE/GPSIMD sub split via pre-wait matmuls).

Math: loss = mean((lfilter(b,a,o) - lfilter(b,a,t))^2)
    = mean(lfilter(b,a,o-t)^2)               [filter is linear]
    = mean(conv(o-t, h)^2)                   [h = impulse response, truncated
                                              to K=128 taps; max pole radius
                                              0.869 -> tail < 2e-8]

Per core (2 rows of T=262144), per row:
  - input arrives in 4 chunk DMAs (natural layout, partition p = contiguous
    2048-sample span), f-order [1536:2048] first (pad source), then
    [0:512], [512:1024], [1024:1536]
  - d = (o - t) cast to bf16, alternating DVE / GPSIMD per chunk
  - transposes as plain bf16 matmuls (lhsT = d block, rhs = I):
    xb data col 128 + 128*tt + p holds 128-sample block (16p + tt)
  - pad cols [0,128): block 16p-1 for col 128+p, taken from the tt=15
    transpose of the first-arriving chunk; col 0 = zeros (row start)
  - conv tile j (B-first pair): psum[jj,n] = sum_i B[i,jj] xb_prev[i,n]
    + sum_i A[i,jj] xb_cur[i,n] (Toeplitz lhsT from h scaled 16x, bf16)
  - square+reduce on ACT (scale=1/16 pre-func) -> per-partition partials;
    host sums / (16*262144)

One sync-wait per instruction (HW limit) is maintained by:
  - funneling the consts DMA through a DVE copy,
  - a tiny "pre-wait" matmul per GPSIMD-subbed chunk that alone carries the
    GPSIMD semaphore wait, so the real transposes carry only a DVE wait,
  - row 0's y tiles in dedicated psum banks (no WAR wait), row 1's tiles
    rotating through 2 banks whose sole psum reader is a DVE copy to SBUF,
  - issuing the output DMA from ACT's own HWDGE queue.
Scheduling hazards learned on HW: psum accumulation pairs must stay
adjacent (B-first gives both matmuls identical readiness), and the final
ACT square must not chase the last matmul's drain window (trailing dummy
matmul + routing the last tile through the SBUF copy path).
"""

import os
import ml_dtypes
import numpy as np

B, T = 16, 262144
NCORES = 8
ROWS_PER_CORE = B // NCORES          # 2
F = 2048                             # free dim of natural layout (T / 128)
K = 128                              # FIR taps
NJ = F // 512                        # 4 conv tiles per row
XBW = 128 + F                        # xb width (128 pad cols + data)
HSCALE = 16.0                        # keep bf16 taps in normal range
NWU = 17                             # PE warmup matmuls (~7us cold)

last_exec_time_ns = None
_CACHE = {}


def _impulse_response(b, a, n):
    """First n samples of the IIR impulse response, float64, DF2T like scipy."""
    b = np.asarray(b, np.float64)
    a = np.asarray(a, np.float64)
    b = b / a[0]
    a = a / a[0]
    order = len(a) - 1
    z = np.zeros(order, np.float64)
    h = np.empty(n, np.float64)
    for i in range(n):
        x = 1.0 if i == 0 else 0.0
        y = b[0] * x + z[0]
        znew = np.empty(order, np.float64)
        znew[: order - 1] = z[1:] + b[1:order] * x - a[1:order] * y
        znew[order - 1] = b[order] * x - a[order] * y
        z = znew
        h[i] = y
    return h


def _toeplitz_lhsts(h):
    """lhsT_A[i,j] = h[j-i] (j>=i), lhsT_B[i,j] = h[128+j-i] (i>j).

    y[128n+j] = sum_{i<=j} h[j-i]*cur[i] + sum_{i>j} h[128+j-i]*prev[i]
    matmul(out, lhsT, rhs): out[j, n] = sum_i lhsT[i, j] * rhs[i, n]
    """
    i = np.arange(K)[:, None]
    j = np.arange(K)[None, :]
    dj = j - i
    A = np.where(dj >= 0, h[np.clip(dj, 0, K - 1)], 0.0)
    Bm = np.where(dj < 0, h[np.clip(K + dj, 0, K - 1)], 0.0)
    return A, Bm


def _drop_vacuous_self_waits(nc):
    """trn2 codegen allows one sync-wait per instruction; Tile sometimes
    attaches a same-engine self-wait alongside a foreign one.  Engine queues
    issue in order and every same-engine op increments the engine sem, so a
    self-wait whose threshold is already guaranteed by queue position is
    droppable."""
    import copy

    prior_incs = {}
    for f in nc.m.functions:
        for bb in f.blocks:
            new_list = []
            for ins in bb.instructions:
                si = ins.sync_info
                if (
                    si is not None
                    and si.on_wait
                    and len(si.on_wait) > 1
                    and "Drain" in type(ins).__name__
                ):
                    waits = list(si.on_wait)
                    for k, w in enumerate(waits[:-1]):
                        pre = copy.deepcopy(ins)
                        pre.name = f"{ins.name}-w{k}"
                        pre.sync_info = copy.deepcopy(si)
                        pre.sync_info.on_wait = [w]
                        pre.sync_info.on_update = []
                        new_list.append(pre)
                    si.on_wait = [waits[-1]]
                new_list.append(ins)
            bb.instructions = new_list
    for f in nc.m.functions:
        for bb in f.blocks:
            for ins in bb.instructions:
                si = ins.sync_info
                if si is None:
                    continue
                waits = list(si.on_wait or [])
                if len(waits) > 1:
                    kept = []
                    for w in waits:
                        name = getattr(w, "ant_name", "") or ""
                        eng = getattr(getattr(ins, "engine", None), "value", "zz")
                        if (
                            name.startswith(eng)
                            and prior_incs.get(name, 0) >= (w.wait_value or 0)
                        ):
                            continue
                        kept.append(w)
                    si.on_wait = kept
                for u in si.on_update or []:
                    name = getattr(u, "ant_name", "") or ""
                    if name:
                        prior_incs[name] = prior_incs.get(name, 0) + (
                            u.update_value or 1
                        )


def _build_bass():
    import concourse.bass as bass
    import concourse.tile as tile
    from concourse import mybir

    dt = mybir.dt
    nc = bass.Bass(trn_type="TRN2")

    ot_h = nc.dram_tensor(
        "ot", [ROWS_PER_CORE, 2, T], dt.float32, kind="ExternalInput"
    )
    # host packs consts partition-major: C_h[p, 128c+f] = (A,B,I)[c][p,f],
    # so the DMA is 128 contiguous 768B descriptors
    C_h = nc.dram_tensor("consts", [K, 3 * K], dt.bfloat16, kind="ExternalInput")
    out_h = nc.dram_tensor(
        "partials", [128, ROWS_PER_CORE * NJ], dt.float32, kind="ExternalOutput"
    )

    # ot4[r, p, s, f] = ot[r, s, 2048p + f]
    ot4 = ot_h[:].rearrange("b s (p f) -> b p s f", p=128)

    # per-row f-chunks in DMA order: chunk 3 (tt 12-15) first so the pad
    # (tt=15) exists before conv tile j=0; then 0, 1, 2.  (f0, f1, j)
    CH = [(1536, 2048, 3), (0, 512, 0), (512, 1024, 1), (1024, 1536, 2)]

    with tile.TileContext(nc) as tc:
        with (
            tc.tile_pool(name="consts", bufs=1) as consts,
            tc.tile_pool(name="io", bufs=2 * NJ) as io_pool,
            tc.tile_pool(name="dpool", bufs=3) as dpool,
            tc.tile_pool(name="xb", bufs=ROWS_PER_CORE) as xbpool,
            tc.tile_pool(name="ptr", bufs=2, space="PSUM") as ptr_pool,
            tc.tile_pool(name="ya", bufs=3, space="PSUM") as ya_pool,
            tc.tile_pool(name="yd", bufs=2, space="PSUM") as yd_pool,
            tc.tile_pool(name="wu", bufs=1, space="PSUM") as wu_pool,
            tc.tile_pool(name="scr", bufs=4) as scr_pool,
            tc.tile_pool(name="outp", bufs=1) as out_pool,
        ):
            # ---- consts first: the DVE funnel copy must run before the
            # per-chunk DVE work starts queuing ----
            c_raw = consts.tile([K, 3, K], dt.bfloat16, tag="Craw")
            nc.sync.dma_start(
                c_raw[:], C_h[:].rearrange("p (c f) -> p c f", c=3)
            )
            c_sb = consts.tile([K, 3, K], dt.bfloat16, tag="C")
            nc.vector.tensor_copy(c_sb[:], c_raw[:])
            A_sb = c_sb[:, 0, :]
            B_sb = c_sb[:, 1, :]
            I_sb = c_sb[:, 2, :]

            # ---- PE HAM warmup: dummy matmuls while the first DMA flies.
            # The warmup bank is the yd pool's first slot: its only accessor
            # is the PE itself, so the slot's later reuse by a conv pair
            # needs no cross-engine WAR wait.
            wu_sb = out_pool.tile([128, 512], dt.bfloat16, tag="wusb")
            nc.vector.memset(wu_sb[:], 0.0)
            wu_ps = wu_pool.tile([128, 512], dt.float32, tag="wu", name="wu")
            for _ in range(NWU):
                nc.tensor.matmul(
                    wu_ps[:], wu_sb[:, 0:128], wu_sb[:], start=True, stop=True
                )

            # ---- input chunk DMAs ----
            io_tiles = {}
            for r in range(ROWS_PER_CORE):
                for ci, (f0, f1, _) in enumerate(CH):
                    t_io = io_pool.tile(
                        [128, 2, 512], dt.float32, tag="ot", name="ot"
                    )
                    nc.sync.dma_start(t_io[:], ot4[r][:, :, f0:f1])
                    io_tiles[(r, ci)] = t_io

            out_sb = out_pool.tile([128, ROWS_PER_CORE * NJ], dt.float32)

            # Row 0's four y tiles get dedicated psum banks (ya, never
            # recycled -> no WAR wait).  Row 1's rotate through 2 yd banks;
            # recycled banks' sole psum reader is a DVE copy to SBUF, so the
            # recycling matmul's WAR wait merges with its DVE data wait.
            tile_ct = [0]

            def y_tile():
                n = tile_ct[0]
                tile_ct[0] += 1
                if n < 3:
                    return ya_pool.tile(
                        [128, 512], dt.float32, tag="ya", name="ya"
                    )
                return yd_pool.tile([128, 512], dt.float32, tag="yd", name="yd")

            def conv_tile(r, j, xb):
                # B first: its deps (prev+cur chunk casts) are a superset of
                # A's, so the pair only becomes ready together and stays
                # adjacent -- split psum accumulation pairs (another group's
                # start/stop in between) corrupt the accumulation on HW.
                py = y_tile()
                n = tile_ct[0] - 1
                nc.tensor.matmul(
                    py[:],
                    B_sb[:],
                    xb[:, 512 * j : 512 * (j + 1)],
                    start=True,
                    stop=False,
                )
                nc.tensor.matmul(
                    py[:],
                    A_sb[:],
                    xb[:, 128 + 512 * j : 128 + 512 * (j + 1)],
                    start=False,
                    stop=True,
                )
                col = NJ * r + j
                acc = out_sb[:, col : col + 1]
                # scr-copy when the yd bank gets recycled later (tiles
                # 3, 4, 5 -> reused by 5, 6, 7) and for the very last tile
                # (guards the ACT-reads-psum-during-drain race window)
                if n < 3 or n == 6:
                    nc.scalar.activation(
                        py[:],
                        py[:],
                        mybir.ActivationFunctionType.Square,
                        scale=1.0 / HSCALE,
                        accum_out=acc,
                    )
                else:
                    scr = scr_pool.tile(
                        [128, 512], dt.bfloat16, tag="scr", name="scr"
                    )
                    nc.vector.tensor_copy(scr[:], py[:])
                    nc.scalar.activation(
                        scr[:],
                        scr[:],
                        mybir.ActivationFunctionType.Square,
                        scale=1.0 / HSCALE,
                        accum_out=acc,
                    )

            chunk_ct = 0
            for r in range(ROWS_PER_CORE):
                xb = xbpool.tile([128, XBW], dt.bfloat16, tag="xb")

                for ci, (f0, f1, j) in enumerate(CH):
                    t_io = io_tiles[(r, ci)]
                    use_gp = False
                    chunk_ct += 1
                    if use_gp:
                        # dedicated slots: GPSIMD never observes the PE sem,
                        # so a recycled slot's WAR would need a second wait
                        d16 = dpool.tile(
                            [128, 512], dt.bfloat16, tag="dg", bufs=4, name="dg"
                        )
                        nc.gpsimd.tensor_sub(
                            d16[:], t_io[:, 0, :], t_io[:, 1, :]
                        )
                    else:
                        d16 = dpool.tile([128, 512], dt.bfloat16, tag="d")
                        nc.vector.tensor_sub(
                            d16[:], t_io[:, 0, :], t_io[:, 1, :]
                        )
                    if use_gp:
                        # tiny pre-wait matmul: alone carries the GPSIMD
                        # wait, so the transposes below carry only their
                        # DVE wait (1 sync-wait HW limit)
                        nc.tensor.matmul(
                            wu_ps[0:1, 0:1],
                            d16[:, 0:1],
                            d16[:, 0:1],
                            start=True,
                            stop=True,
                        )

                    ptr = ptr_pool.tile([128, 512], dt.float32, tag="tr")
                    for q in range(4):
                        nc.tensor.matmul(
                            ptr[:, 128 * q : 128 * (q + 1)],
                            d16[:, 128 * q : 128 * (q + 1)],
                            I_sb[:],
                            start=True,
                            stop=True,
                        )
                    nc.vector.tensor_copy(
                        xb[:, 128 + f0 : 128 + f1], ptr[:]
                    )

                    if j == 3:
                        # first-arriving chunk carries tile tt=15 at
                        # ptr[:, 384:512]: pad col p = block 16p-1 = tt15
                        # col p-1; col 0 = zeros (zero state at row start)
                        nc.vector.memset(xb[:, 0:1], 0.0)
                        nc.vector.tensor_copy(
                            xb[:, 1:128], ptr[:, 384 : 384 + 127]
                        )
                    else:
                        conv_tile(r, j, xb)
                        if j == 2:
                            # chunk 2 is the row's last arrival; tile 3's
                            # B operand (cols 1536:2048) is now complete
                            conv_tile(r, 3, xb)
                            # trailing dummy: pushes the last squares' PE
                            # sem threshold past the drain window
                            nc.tensor.matmul(
                                wu_ps[:, 0:128],
                                wu_sb[:, 0:128],
                                wu_sb[:, 0:128],
                                start=True,
                                stop=True,
                            )

            # issue from ACT's HWDGE queue: the dep on ACT's accum writes is
            # implicit in program order
            nc.scalar.dma_start(out_h[:], out_sb[:])

    _drop_vacuous_self_waits(nc)
    return nc


def kernel(output, target, b, a):
    global last_exec_time_ns
    from concourse.bass_utils import run_bass_kernel_spmd

    output = np.asarray(output, np.float32)
    target = np.asarray(target, np.float32)

    if "nc" not in _CACHE:
        _CACHE["nc"] = _build_bass()
    nc = _CACHE["nc"]

    h = _impulse_response(np.asarray(b, np.float64), np.asarray(a, np.float64), K)
    A_m, B_m = _toeplitz_lhsts(h * HSCALE)
    # partition-major packing: consts[p, 128c+f] = (A,B,I)[c][p,f]
    consts = np.ascontiguousarray(
        np.stack([A_m, B_m, np.eye(K)]).transpose(1, 0, 2).reshape(K, 3 * K)
    ).astype(ml_dtypes.bfloat16)

    ot = np.stack([output, target], axis=1)  # [B, 2, T]
    in_maps = []
    for c in range(NCORES):
        rows = slice(c * ROWS_PER_CORE, (c + 1) * ROWS_PER_CORE)
        in_maps.append(
            {
                "ot": np.ascontiguousarray(ot[rows]),
                "consts": consts,
            }
        )

    res = run_bass_kernel_spmd(
        nc,
        in_maps,
        core_ids=list(range(NCORES)),
        trace=bool(int(os.environ.get("LP_TRACE", "0"))),
    )
    last_exec_time_ns = res.exec_time_ns

    total = np.float64(0.0)
    for r in res.results:
        total += r["partials"].astype(np.float64).sum()
    # squares are descaled by 1/HSCALE inside the ACT (scale applies pre-func)
    return np.float32(total / (B * T))


# revision 58
# speedup vs baseline: 1.0107x; 1.0107x over previous
"""LowPassMSELoss Trainium2 kernel (v4: chunked DMA pipeline, bf16 PE path,
PE HAM warmup, DVE/GPSIMD sub split via pre-wait matmuls).

Math: loss = mean((lfilter(b,a,o) - lfilter(b,a,t))^2)
    = mean(lfilter(b,a,o-t)^2)               [filter is linear]
    = mean(conv(o-t, h)^2)                   [h = impulse response, truncated
                                              to K=128 taps; max pole radius
                                              0.869 -> tail < 2e-8]

Per core (2 rows of T=262144), per row:
  - input arrives in 4 chunk DMAs (natural layout, partition p = contiguous
    2048-sample span), f-order [1536:2048] first (pad source), then
    [0:512], [512:1024], [1024:1536]
  - d = (o - t) cast to bf16, alternating DVE / GPSIMD per chunk
  - transposes as plain bf16 matmuls (lhsT = d block, rhs = I):
    xb data col 128 + 128*tt + p holds 128-sample block (16p + tt)
  - pad cols [0,128): block 16p-1 for col 128+p, taken from the tt=15
    transpose of the first-arriving chunk; col 0 = zeros (row start)
  - conv tile j (B-first pair): psum[jj,n] = sum_i B[i,jj] xb_prev[i,n]
    + sum_i A[i,jj] xb_cur[i,n] (Toeplitz lhsT from h scaled 16x, bf16)
  - square+reduce on ACT (scale=1/16 pre-func) -> per-partition partials;
    host sums / (16*262144)

One sync-wait per instruction (HW limit) is maintained by:
  - funneling the consts DMA through a DVE copy,
  - a tiny "pre-wait" matmul per GPSIMD-subbed chunk that alone carries the
    GPSIMD semaphore wait, so the real transposes carry only a DVE wait,
  - row 0's y tiles in dedicated psum banks (no WAR wait), row 1's tiles
    rotating through 2 banks whose sole psum reader is a DVE copy to SBUF,
  - issuing the output DMA from ACT's own HWDGE queue.
Scheduling hazards learned on HW: psum accumulation pairs must stay
adjacent (B-first gives both matmuls identical readiness), and the final
ACT square must not chase the last matmul's drain window (trailing dummy
matmul + routing the last tile through the SBUF copy path).
"""

import os
import ml_dtypes
import numpy as np

B, T = 16, 262144
NCORES = 8
ROWS_PER_CORE = B // NCORES          # 2
F = 2048                             # free dim of natural layout (T / 128)
K = 128                              # FIR taps
NJ = F // 512                        # 4 conv tiles per row
XBW = 128 + F                        # xb width (128 pad cols + data)
HSCALE = 16.0                        # keep bf16 taps in normal range
NWU = 17                             # PE warmup matmuls (~7us cold)

last_exec_time_ns = None
_CACHE = {}


def _impulse_response(b, a, n):
    """First n samples of the IIR impulse response, float64, DF2T like scipy."""
    b = np.asarray(b, np.float64)
    a = np.asarray(a, np.float64)
    b = b / a[0]
    a = a / a[0]
    order = len(a) - 1
    z = np.zeros(order, np.float64)
    h = np.empty(n, np.float64)
    for i in range(n):
        x = 1.0 if i == 0 else 0.0
        y = b[0] * x + z[0]
        znew = np.empty(order, np.float64)
        znew[: order - 1] = z[1:] + b[1:order] * x - a[1:order] * y
        znew[order - 1] = b[order] * x - a[order] * y
        z = znew
        h[i] = y
    return h


def _toeplitz_lhsts(h):
    """lhsT_A[i,j] = h[j-i] (j>=i), lhsT_B[i,j] = h[128+j-i] (i>j).

    y[128n+j] = sum_{i<=j} h[j-i]*cur[i] + sum_{i>j} h[128+j-i]*prev[i]
    matmul(out, lhsT, rhs): out[j, n] = sum_i lhsT[i, j] * rhs[i, n]
    """
    i = np.arange(K)[:, None]
    j = np.arange(K)[None, :]
    dj = j - i
    A = np.where(dj >= 0, h[np.clip(dj, 0, K - 1)], 0.0)
    Bm = np.where(dj < 0, h[np.clip(K + dj, 0, K - 1)], 0.0)
    return A, Bm


def _drop_vacuous_self_waits(nc):
    """trn2 codegen allows one sync-wait per instruction; Tile sometimes
    attaches a same-engine self-wait alongside a foreign one.  Engine queues
    issue in order and every same-engine op increments the engine sem, so a
    self-wait whose threshold is already guaranteed by queue position is
    droppable."""
    import copy

    prior_incs = {}
    for f in nc.m.functions:
        for bb in f.blocks:
            new_list = []
            for ins in bb.instructions:
                si = ins.sync_info
                if (
                    si is not None
                    and si.on_wait
                    and len(si.on_wait) > 1
                    and "Drain" in type(ins).__name__
                ):
                    waits = list(si.on_wait)
                    for k, w in enumerate(waits[:-1]):
                        pre = copy.deepcopy(ins)
                        pre.name = f"{ins.name}-w{k}"
                        pre.sync_info = copy.deepcopy(si)
                        pre.sync_info.on_wait = [w]
                        pre.sync_info.on_update = []
                        new_list.append(pre)
                    si.on_wait = [waits[-1]]
                new_list.append(ins)
            bb.instructions = new_list
    for f in nc.m.functions:
        for bb in f.blocks:
            for ins in bb.instructions:
                si = ins.sync_info
                if si is None:
                    continue
                waits = list(si.on_wait or [])
                if len(waits) > 1:
                    kept = []
                    for w in waits:
                        name = getattr(w, "ant_name", "") or ""
                        eng = getattr(getattr(ins, "engine", None), "value", "zz")
                        if (
                            name.startswith(eng)
                            and prior_incs.get(name, 0) >= (w.wait_value or 0)
                        ):
                            continue
                        kept.append(w)
                    si.on_wait = kept
                for u in si.on_update or []:
                    name = getattr(u, "ant_name", "") or ""
                    if name:
                        prior_incs[name] = prior_incs.get(name, 0) + (
                            u.update_value or 1
                        )


def _build_bass():
    import concourse.bass as bass
    import concourse.tile as tile
    from concourse import mybir

    dt = mybir.dt
    nc = bass.Bass(trn_type="TRN2")

    ot_h = nc.dram_tensor(
        "ot", [ROWS_PER_CORE, 2, T], dt.float32, kind="ExternalInput"
    )
    # host packs consts partition-major: C_h[p, 128c+f] = (A,B,I)[c][p,f],
    # so the DMA is 128 contiguous 768B descriptors
    C_h = nc.dram_tensor("consts", [K, 3 * K], dt.bfloat16, kind="ExternalInput")
    out_h = nc.dram_tensor(
        "partials", [128, ROWS_PER_CORE * NJ], dt.float32, kind="ExternalOutput"
    )

    # ot4[r, p, s, f] = ot[r, s, 2048p + f]
    ot4 = ot_h[:].rearrange("b s (p f) -> b p s f", p=128)

    # per-row f-chunks in DMA order: chunk 3 (tt 12-15) first so the pad
    # (tt=15) exists before conv tile j=0; then 0, 1, 2.  (f0, f1, j)
    CH = [(1536, 2048, 3), (0, 512, 0), (512, 1024, 1), (1024, 1536, 2)]

    with tile.TileContext(nc) as tc:
        with (
            tc.tile_pool(name="consts", bufs=1) as consts,
            tc.tile_pool(name="io", bufs=2 * NJ) as io_pool,
            tc.tile_pool(name="dpool", bufs=3) as dpool,
            tc.tile_pool(name="xb", bufs=ROWS_PER_CORE) as xbpool,
            tc.tile_pool(name="ptr", bufs=2, space="PSUM") as ptr_pool,
            tc.tile_pool(name="ya", bufs=3, space="PSUM") as ya_pool,
            tc.tile_pool(name="yd", bufs=2, space="PSUM") as yd_pool,
            tc.tile_pool(name="wu", bufs=1, space="PSUM") as wu_pool,
            tc.tile_pool(name="scr", bufs=4) as scr_pool,
            tc.tile_pool(name="outp", bufs=1) as out_pool,
        ):
            # ---- consts first: the DVE funnel copy must run before the
            # per-chunk DVE work starts queuing ----
            c_raw = consts.tile([K, 3, K], dt.bfloat16, tag="Craw")
            nc.sync.dma_start(
                c_raw[:], C_h[:].rearrange("p (c f) -> p c f", c=3)
            )
            c_sb = consts.tile([K, 3, K], dt.bfloat16, tag="C")
            nc.vector.tensor_copy(c_sb[:], c_raw[:])
            A_sb = c_sb[:, 0, :]
            B_sb = c_sb[:, 1, :]
            I_sb = c_sb[:, 2, :]

            # ---- PE HAM warmup: dummy matmuls while the first DMA flies.
            # The warmup bank is the yd pool's first slot: its only accessor
            # is the PE itself, so the slot's later reuse by a conv pair
            # needs no cross-engine WAR wait.
            wu_sb = out_pool.tile([128, 512], dt.bfloat16, tag="wusb")
            nc.vector.memset(wu_sb[:], 0.0)
            wu_ps = wu_pool.tile([128, 512], dt.float32, tag="wu", name="wu")
            for _ in range(NWU):
                nc.tensor.matmul(
                    wu_ps[:], wu_sb[:, 0:128], wu_sb[:], start=True, stop=True
                )

            # ---- input chunk DMAs ----
            io_tiles = {}
            for r in range(ROWS_PER_CORE):
                for ci, (f0, f1, _) in enumerate(CH):
                    t_io = io_pool.tile(
                        [128, 2, 512], dt.float32, tag="ot", name="ot"
                    )
                    nc.sync.dma_start(t_io[:], ot4[r][:, :, f0:f1])
                    io_tiles[(r, ci)] = t_io

            out_sb = out_pool.tile([128, ROWS_PER_CORE * NJ], dt.float32)

            # Row 0's four y tiles get dedicated psum banks (ya, never
            # recycled -> no WAR wait).  Row 1's rotate through 2 yd banks;
            # recycled banks' sole psum reader is a DVE copy to SBUF, so the
            # recycling matmul's WAR wait merges with its DVE data wait.
            tile_ct = [0]

            def y_tile():
                n = tile_ct[0]
                tile_ct[0] += 1
                if n < 3:
                    return ya_pool.tile(
                        [128, 512], dt.float32, tag="ya", name="ya"
                    )
                return yd_pool.tile([128, 512], dt.float32, tag="yd", name="yd")

            def conv_tile(r, j, xb):
                # B first: its deps (prev+cur chunk casts) are a superset of
                # A's, so the pair only becomes ready together and stays
                # adjacent -- split psum accumulation pairs (another group's
                # start/stop in between) corrupt the accumulation on HW.
                py = y_tile()
                n = tile_ct[0] - 1
                nc.tensor.matmul(
                    py[:],
                    B_sb[:],
                    xb[:, 512 * j : 512 * (j + 1)],
                    start=True,
                    stop=False,
                )
                nc.tensor.matmul(
                    py[:],
                    A_sb[:],
                    xb[:, 128 + 512 * j : 128 + 512 * (j + 1)],
                    start=False,
                    stop=True,
                )
                col = NJ * r + j
                acc = out_sb[:, col : col + 1]
                # scr-copy when the yd bank gets recycled later (tiles
                # 3, 4, 5 -> reused by 5, 6, 7) and for the very last tile
                # (guards the ACT-reads-psum-during-drain race window)
                if n < 3 or n == 6:
                    nc.scalar.activation(
                        py[:],
                        py[:],
                        mybir.ActivationFunctionType.Square,
                        scale=1.0 / HSCALE,
                        accum_out=acc,
                    )
                else:
                    scr = scr_pool.tile(
                        [128, 512], dt.bfloat16, tag="scr", name="scr"
                    )
                    nc.vector.tensor_copy(scr[:], py[:])
                    nc.scalar.activation(
                        scr[:],
                        scr[:],
                        mybir.ActivationFunctionType.Square,
                        scale=1.0 / HSCALE,
                        accum_out=acc,
                    )

            chunk_ct = 0
            for r in range(ROWS_PER_CORE):
                xb = xbpool.tile([128, XBW], dt.bfloat16, tag="xb")

                for ci, (f0, f1, j) in enumerate(CH):
                    t_io = io_tiles[(r, ci)]
                    use_gp = chunk_ct in (1, 2, 4, 5)
                    chunk_ct += 1
                    if use_gp:
                        # dedicated slots: GPSIMD never observes the PE sem,
                        # so a recycled slot's WAR would need a second wait
                        d16 = dpool.tile(
                            [128, 512], dt.bfloat16, tag="dg", bufs=4, name="dg"
                        )
                        nc.gpsimd.tensor_sub(
                            d16[:], t_io[:, 0, :], t_io[:, 1, :]
                        )
                    else:
                        d16 = dpool.tile([128, 512], dt.bfloat16, tag="d")
                        nc.vector.tensor_sub(
                            d16[:], t_io[:, 0, :], t_io[:, 1, :]
                        )
                    if use_gp:
                        # tiny pre-wait matmul: alone carries the GPSIMD
                        # wait, so the transposes below carry only their
                        # DVE wait (1 sync-wait HW limit)
                        nc.tensor.matmul(
                            wu_ps[0:1, 0:1],
                            d16[:, 0:1],
                            d16[:, 0:1],
                            start=True,
                            stop=True,
                        )

                    ptr = ptr_pool.tile([128, 512], dt.float32, tag="tr")
                    for q in range(4):
                        nc.tensor.matmul(
                            ptr[:, 128 * q : 128 * (q + 1)],
                            d16[:, 128 * q : 128 * (q + 1)],
                            I_sb[:],
                            start=True,
                            stop=True,
                        )
                    nc.vector.tensor_copy(
                        xb[:, 128 + f0 : 128 + f1], ptr[:]
                    )

                    if j == 3:
                        # first-arriving chunk carries tile tt=15 at
                        # ptr[:, 384:512]: pad col p = block 16p-1 = tt15
                        # col p-1; col 0 = zeros (zero state at row start)
                        nc.vector.memset(xb[:, 0:1], 0.0)
                        nc.vector.tensor_copy(
                            xb[:, 1:128], ptr[:, 384 : 384 + 127]
                        )
                    else:
                        conv_tile(r, j, xb)
                        if j == 2:
                            # chunk 2 is the row's last arrival; tile 3's
                            # B operand (cols 1536:2048) is now complete
                            conv_tile(r, 3, xb)
                            # trailing dummy: pushes the last squares' PE
                            # sem threshold past the drain window
                            nc.tensor.matmul(
                                wu_ps[:, 0:128],
                                wu_sb[:, 0:128],
                                wu_sb[:, 0:128],
                                start=True,
                                stop=True,
                            )

            # issue from ACT's HWDGE queue: the dep on ACT's accum writes is
            # implicit in program order
            nc.scalar.dma_start(out_h[:], out_sb[:])

    _drop_vacuous_self_waits(nc)
    return nc


def kernel(output, target, b, a):
    global last_exec_time_ns
    from concourse.bass_utils import run_bass_kernel_spmd

    output = np.asarray(output, np.float32)
    target = np.asarray(target, np.float32)

    if "nc" not in _CACHE:
        _CACHE["nc"] = _build_bass()
    nc = _CACHE["nc"]

    h = _impulse_response(np.asarray(b, np.float64), np.asarray(a, np.float64), K)
    A_m, B_m = _toeplitz_lhsts(h * HSCALE)
    # partition-major packing: consts[p, 128c+f] = (A,B,I)[c][p,f]
    consts = np.ascontiguousarray(
        np.stack([A_m, B_m, np.eye(K)]).transpose(1, 0, 2).reshape(K, 3 * K)
    ).astype(ml_dtypes.bfloat16)

    ot = np.stack([output, target], axis=1)  # [B, 2, T]
    in_maps = []
    for c in range(NCORES):
        rows = slice(c * ROWS_PER_CORE, (c + 1) * ROWS_PER_CORE)
        in_maps.append(
            {
                "ot": np.ascontiguousarray(ot[rows]),
                "consts": consts,
            }
        )

    res = run_bass_kernel_spmd(
        nc,
        in_maps,
        core_ids=list(range(NCORES)),
        trace=bool(int(os.environ.get("LP_TRACE", "0"))),
    )
    last_exec_time_ns = res.exec_time_ns

    total = np.float64(0.0)
    for r in res.results:
        total += r["partials"].astype(np.float64).sum()
    # squares are descaled by 1/HSCALE inside the ACT (scale applies pre-func)
    return np.float32(total / (B * T))
